# revision 16
# baseline (speedup 1.0000x reference)
"""Trainium2 Bass kernel for nn_CaduceusEmbeddingsSTFT.

out[b, t, :] = concat(emb_table[ids[b, t]],
                      proj(|STFT(onehot(ids[b]))| upsampled at frame f(t)))

Structure exploited:
  * nearest upsampling -> only 129 distinct STFT frame rows per batch; the
    (8192 x 2064) @ (2064 x 154) projection collapses to (129 x 2064) @
    (2064 x 154) plus a row broadcast.
  * STFT of one-hot signals: windowed frames are one-hot masks, so
    spec = onehot_frames @ (window * DFT) as matmuls (cos / sin).
  * embedding lookup and frame broadcast are one-hot matmuls on the PE.

All matmuls run in bf16 (fp32 matmuls execute as two passes and tend to
stay HAM-throttled). Precision is preserved by exact hi+lo bf16 splits:
one-hot operands are exact in bf16; the other side is split so
x = hi + lo with both parts bf16 and the product accumulated in fp32
PSUM (omitted lo*lo cross terms are ~2^-18 relative).

Sharding: 8 cores = 4 batches x 2 sequence halves; each core computes a
(4096, 512) output shard; boundary frame recomputed by both halves.
"""

import numpy as np

V = 16
D_EMB = 358
D_STFT = 154
NFFT = 256
HOP = 64
NFREQ = 129
B, L = 4, 8192
LH = L // 2  # 4096 rows per core
F = 65  # frames per core (inclusive overlap frame)
VF = V * F  # 1040
DM = 512
NCORES = 8
NT = LH // 128  # 32 output tiles per core
NQ = NT // 4  # q-groups of 4 tiles
# (start, size) chunks over the VF axis; multiples of F so projection
# lhsT slices [:, v*F:(v+1)*F] never cross a chunk boundary.
CHUNKS = [(0, 7 * F), (7 * F, 7 * F), (14 * F, 2 * F)]

_PROG = None
LAST_RESULT = None  # BassKernelResults of the most recent run (for harnesses)


def _build_program():
    import concourse.mybir as mybir
    import concourse.tile as tile
    from concourse import bacc

    f32 = mybir.dt.float32
    bf16 = mybir.dt.bfloat16
    i8 = mybir.dt.int8
    AO = mybir.AluOpType
    AF = mybir.ActivationFunctionType

    nc = bacc.Bacc("TRN2", target_bir_lowering=False, debug=False,
                   num_devices=NCORES)

    h_frames = nc.dram_tensor("h_frames", [128, 2 * F], i8, kind="ExternalInput")
    h_emb = nc.dram_tensor("h_emb", [128, LH // 4], i8, kind="ExternalInput")
    vfr = nc.dram_tensor("vfr", [128, V], i8, kind="ExternalInput")
    vemb = nc.dram_tensor("vemb", [128, 1], i8, kind="ExternalInput")
    # windowed DFT: [c][hi|lo][cos k=0..127, nyq cos, sin k=0..127, pad]
    cw = nc.dram_tensor("cw", [128, 4 * 2 * NFREQ], bf16, kind="ExternalInput")
    bsel = nc.dram_tensor("bsel", [F + 1, LH], bf16, kind="ExternalInput")
    wproj = nc.dram_tensor("wproj", [128, 2 * V * D_STFT], bf16,
                           kind="ExternalInput")
    wnyq = nc.dram_tensor("wnyq", [V, 2 * D_STFT], bf16, kind="ExternalInput")
    pbias = nc.dram_tensor("pbias", [2, D_STFT], bf16, kind="ExternalInput")
    embrep = nc.dram_tensor("embrep", [128, D_EMB], bf16, kind="ExternalInput")
    out = nc.dram_tensor("out", [LH, DM], f32, kind="ExternalOutput")

    CWW = 2 * NFREQ  # 258: per-(c,part) block width in cw

    with tile.TileContext(nc) as tc:
        with (
            tc.tile_pool(name="consts", bufs=1) as cpool,
            tc.tile_pool(name="work", bufs=1) as wpool,
            tc.tile_pool(name="tmp", bufs=2) as tpool,
            tc.tile_pool(name="oemb", bufs=2) as oepool,
            tc.tile_pool(name="ostft", bufs=2) as ospool,
        ):
            # ---- const loads (small ones first: they gate the pipeline) ----
            HE = cpool.tile([128, LH // 4], i8, tag="he")
            nc.sync.dma_start(out=HE[:], in_=h_emb[:])
            VEMB = cpool.tile([128, 1], i8, tag="vemb")
            nc.sync.dma_start(out=VEMB[:], in_=vemb[:])
            ER = cpool.tile([128, D_EMB], bf16, tag="er")
            nc.sync.dma_start(out=ER[:], in_=embrep[:])
            HF = cpool.tile([128, 2 * F], i8, tag="hf")
            nc.sync.dma_start(out=HF[:], in_=h_frames[:])
            VFR = cpool.tile([128, V], i8, tag="vfr")
            nc.sync.dma_start(out=VFR[:], in_=vfr[:])
            CW = cpool.tile([128, 4 * CWW], bf16, tag="cw")
            nc.sync.dma_start(out=CW[:], in_=cw[:])
            WP = cpool.tile([128, 2 * V * D_STFT], bf16, tag="wp")
            nc.sync.dma_start(out=WP[:], in_=wproj[:])
            WN = cpool.tile([V, 2 * D_STFT], bf16, tag="wn")
            nc.sync.dma_start(out=WN[:], in_=wnyq[:])
            BS = cpool.tile([F + 1, LH], bf16, tag="bs")
            nc.sync.dma_start(out=BS[:], in_=bsel[:])

            # ---- one-hot builds (bf16 out: 0/1 exact) -----------------------
            OHE = wpool.tile([128, LH // 4], bf16, tag="ohe")
            nc.vector.tensor_tensor(
                out=OHE[:], in0=HE[:], in1=VEMB[:].to_broadcast([128, LH // 4]),
                op=AO.is_equal)
            OHF = []
            for c in range(2):
                t = wpool.tile([128, VF], bf16, tag=f"ohf{c}")
                in0 = (HF[:, c * F:(c + 1) * F]
                       .rearrange("p (one f) -> p one f", one=1)
                       .to_broadcast([128, V, F]))
                in1 = (VFR[:].rearrange("p (v one) -> p v one", one=1)
                       .to_broadcast([128, V, F]))
                nc.vector.tensor_tensor(
                    out=t[:].rearrange("p (v f) -> p v f", v=V),
                    in0=in0, in1=in1, op=AO.is_equal)
                OHF.append(t)

            # ---- DFT matmuls (PE) -------------------------------------------
            # lhsT blocks in CW: [c*2][cos|nyq] hi, [c*2+1] lo
            MAG = wpool.tile([128, VF], f32, tag="mag")
            MAGN = wpool.tile([2, VF], f32, tag="magn")
            dft_psums = []
            with (
                tc.tile_pool(name="psum_re", bufs=1, space="PSUM") as pre,
                tc.tile_pool(name="psum_im", bufs=1, space="PSUM") as pim,
                tc.tile_pool(name="psum_ny", bufs=1, space="PSUM") as pny,
            ):
                for ci, (c0, cn) in enumerate(CHUNKS):
                    re = pre.tile([128, cn], f32, tag="re")
                    im = pim.tile([128, cn], f32, tag="im")
                    ny = pny.tile([2, cn], f32, tag="ny")
                    first = True
                    for c in range(2):
                        for part in range(2):  # hi, lo
                            cb = (2 * c + part) * CWW
                            rhs = OHF[c][:, c0:c0 + cn]
                            nc.tensor.matmul(
                                out=re[:], lhsT=CW[:, cb:cb + 128], rhs=rhs,
                                start=first, stop=(c == 1 and part == 1))
                            nc.tensor.matmul(
                                out=im[:],
                                lhsT=CW[:, cb + NFREQ:cb + NFREQ + 128],
                                rhs=rhs,
                                start=first, stop=(c == 1 and part == 1))
                            # nyq: [cos128 | sin128(=0 -> borrow col for pad)]
                            nc.tensor.matmul(
                                out=ny[:1, :], lhsT=CW[:, cb + 128:cb + 129],
                                rhs=rhs,
                                start=first, stop=(c == 1 and part == 1))
                            first = False
                    dft_psums.append((re, im, ny))

                # ---- emb pipeline (PE + copies + early stores) --------------
                with tc.tile_pool(name="psum_emb", bufs=4, space="PSUM") as pemb:
                    for q in range(NQ):
                        oe = oepool.tile([128, 4 * D_EMB], f32, tag="oe")
                        for a in range(4):
                            po = pemb.tile([128, D_EMB], f32, tag="pe")
                            nc.tensor.matmul(
                                out=po[:],
                                lhsT=OHE[32 * a:32 * a + V,
                                         q * 128:(q + 1) * 128],
                                rhs=ER[32 * a:32 * a + V, :],
                                start=True, stop=True,
                                tile_position=(32 * a, 0))
                            sl = oe[:, a * D_EMB:(a + 1) * D_EMB]
                            if a % 2 == 0:
                                nc.vector.tensor_copy(out=sl, in_=po[:])
                            else:
                                nc.scalar.copy(out=sl, in_=po[:])
                        # one DMA for 4 tiles: rows 512q..512q+511, cols 0:358
                        # (src stays partition-major; dest iterates (p, a, e))
                        nc.sync.dma_start(
                            out=out[q * 512:(q + 1) * 512, :D_EMB]
                            .rearrange("(a p) e -> p a e", p=128),
                            in_=oe[:].rearrange("p (a e) -> p a e", a=4))

                # ---- magnitudes (DVE/ACT), overlapping emb PE work ----------
                for ci, (c0, cn) in enumerate(CHUNKS):
                    re, im, ny = dft_psums[ci]
                    t1 = tpool.tile([128, cn], f32, tag="sq1")
                    t2 = tpool.tile([128, cn], f32, tag="sq2")
                    nc.scalar.square(out=t1[:], in_=re[:])
                    nc.scalar.square(out=t2[:], in_=im[:])
                    nc.vector.tensor_tensor(out=t1[:], in0=t1[:], in1=t2[:],
                                            op=AO.add)
                    nc.scalar.sqrt(out=MAG[:, c0:c0 + cn], in_=t1[:])
                    nc.scalar.activation(MAGN[:1, c0:c0 + cn], ny[:1, :],
                                         AF.Abs)

            # ---- bf16 hi/lo splits of MAG and MAGN --------------------------
            MAGH = wpool.tile([128, VF], bf16, tag="magh")
            MAGL = wpool.tile([128, VF], bf16, tag="magl")
            nc.vector.tensor_copy(out=MAGH[:], in_=MAG[:])
            nc.vector.tensor_tensor(out=MAGL[:], in0=MAG[:], in1=MAGH[:],
                                    op=AO.subtract)
            NYH = wpool.tile([1, VF], bf16, tag="nyh")
            NYL = wpool.tile([1, VF], bf16, tag="nyl")
            nc.vector.tensor_copy(out=NYH[:], in_=MAGN[:1, :])
            nc.vector.tensor_tensor(out=NYL[:], in0=MAGN[:1, :], in1=NYH[:],
                                    op=AO.subtract)
            NYQTH = wpool.tile([V, F], bf16, tag="nyqth")
            NYQTL = wpool.tile([V, F], bf16, tag="nyqtl")
            nc.sync.dma_start(out=NYQTH[:, :], in_=NYH[:, :])
            nc.sync.dma_start(out=NYQTL[:, :], in_=NYL[:, :])

            # ---- projection: S = sum_{v,k} mag * W  (hi/lo cross terms) -----
            with tc.tile_pool(name="psum_s", bufs=1, space="PSUM") as psp:
                S = psp.tile([F, D_STFT], f32, tag="s")
                for v in range(V):
                    mh = MAGH[:, v * F:(v + 1) * F]
                    ml = MAGL[:, v * F:(v + 1) * F]
                    wh = WP[:, (2 * v) * D_STFT:(2 * v + 1) * D_STFT]
                    wl = WP[:, (2 * v + 1) * D_STFT:(2 * v + 2) * D_STFT]
                    nc.tensor.matmul(out=S[:], lhsT=mh, rhs=wh,
                                     start=(v == 0), stop=False)
                    nc.tensor.matmul(out=S[:], lhsT=mh, rhs=wl,
                                     start=False, stop=False)
                    nc.tensor.matmul(out=S[:], lhsT=ml, rhs=wh,
                                     start=False, stop=False)
                nc.tensor.matmul(out=S[:], lhsT=NYQTH[:],
                                 rhs=WN[:, :D_STFT], start=False, stop=False)
                nc.tensor.matmul(out=S[:], lhsT=NYQTH[:],
                                 rhs=WN[:, D_STFT:], start=False, stop=False)
                nc.tensor.matmul(out=S[:], lhsT=NYQTL[:],
                                 rhs=WN[:, :D_STFT], start=False, stop=True)
                # S split to bf16 hi/lo; row F carries proj_b (host hi/lo)
                SH = wpool.tile([F + 1, D_STFT], bf16, tag="sh")
                SL = wpool.tile([F + 1, D_STFT], bf16, tag="sl")
                nc.vector.tensor_copy(out=SH[:F, :], in_=S[:])
                nc.vector.tensor_tensor(out=SL[:F, :], in0=S[:],
                                        in1=SH[:F, :], op=AO.subtract)
                nc.sync.dma_start(out=SH[F:F + 1, :], in_=pbias[0:1, :])
                nc.sync.dma_start(out=SL[F:F + 1, :], in_=pbias[1:2, :])

            # ---- stft part of output: B-select @ (SH + SL) ------------------
            with tc.tile_pool(name="psum_stft", bufs=4, space="PSUM") as pstft:
                for q in range(NQ):
                    os_ = ospool.tile([128, 4 * D_STFT], f32, tag="os")
                    for a in range(4):
                        ti = q * 4 + a
                        ps = pstft.tile([128, D_STFT], f32, tag="ps")
                        lhsT = BS[:, ti * 128:(ti + 1) * 128]
                        nc.tensor.matmul(out=ps[:], lhsT=lhsT, rhs=SH[:],
                                         start=True, stop=False)
                        nc.tensor.matmul(out=ps[:], lhsT=lhsT, rhs=SL[:],
                                         start=False, stop=True)
                        sl = os_[:, a * D_STFT:(a + 1) * D_STFT]
                        if a % 2 == 0:
                            nc.scalar.copy(out=sl, in_=ps[:])
                        else:
                            nc.vector.tensor_copy(out=sl, in_=ps[:])
                    nc.sync.dma_start(
                        out=out[q * 512:(q + 1) * 512, D_EMB:DM]
                        .rearrange("(a p) e -> p a e", p=128),
                        in_=os_[:].rearrange("p (a e) -> p a e", a=4))

    nc.finalize()
    return nc


def _split_bf16(x):
    import ml_dtypes

    hi = x.astype(ml_dtypes.bfloat16)
    lo = (x - hi.astype(np.float32)).astype(ml_dtypes.bfloat16)
    return hi, lo


def _host_consts():
    import ml_dtypes

    bf16 = ml_dtypes.bfloat16
    n = np.arange(NFFT)
    window = 0.5 - 0.5 * np.cos(2.0 * np.pi * n / NFFT)
    k = np.arange(NFREQ)
    ang = 2.0 * np.pi * np.outer(n, k) / NFFT  # (256, 129)
    wcos = (window[:, None] * np.cos(ang)).astype(np.float32)
    wsin = (window[:, None] * np.sin(ang)).astype(np.float32)
    CWW = 2 * NFREQ
    cwf = np.zeros((128, 4 * CWW), np.float32)
    for c in range(2):
        rows = slice(c * 128, (c + 1) * 128)
        # block layout per (c, part): [cos k0..k127, cos k128, sin k0..k127, 0]
        blk = np.zeros((128, CWW), np.float32)
        blk[:, :128] = wcos[rows, :128]
        blk[:, 128] = wcos[rows][:, 128]  # nyquist cos column
        blk[:, NFREQ:NFREQ + 128] = wsin[rows, :128]
        hi, lo = _split_bf16(blk)
        cwf[:, (2 * c) * CWW:(2 * c + 1) * CWW] = hi.astype(np.float32)
        cwf[:, (2 * c + 1) * CWW:(2 * c + 2) * CWW] = lo.astype(np.float32)
    cw = cwf.astype(bf16)

    vfr = np.broadcast_to(np.arange(V, dtype=np.int8), (128, V)).copy()
    vemb = (np.arange(128, dtype=np.int8) % 32).reshape(128, 1).copy()
    return cw, vfr, vemb


def _bsel_for_half(h):
    import ml_dtypes

    t = np.arange(LH)
    fglob = (129 * (t + LH * h)) >> 13
    floc = fglob - 64 * h
    bs = np.zeros((F + 1, LH), np.float32)
    bs[floc, t] = 1.0
    bs[F, :] = 1.0  # ones row: adds proj_b (rows F of SH/SL)
    return bs.astype(ml_dtypes.bfloat16)


def kernel(input_ids, emb_table, proj_w, proj_b):
    global _PROG, LAST_RESULT
    import ml_dtypes

    from concourse.bass_utils import run_bass_kernel_spmd

    bf16 = ml_dtypes.bfloat16
    ids = np.asarray(input_ids).astype(np.int32)
    emb = np.asarray(emb_table).astype(np.float32)
    pw = np.asarray(proj_w).astype(np.float32)
    pb = np.asarray(proj_b).astype(np.float32)

    cw, vfr, vemb = _host_consts()

    # proj_w rows are indexed by i = k*V + v (freq-major); interleave hi/lo
    wproj = np.zeros((128, 2 * V * D_STFT), np.float32)
    for v in range(V):
        hi, lo = _split_bf16(pw[np.arange(128) * V + v])
        wproj[:, (2 * v) * D_STFT:(2 * v + 1) * D_STFT] = hi.astype(np.float32)
        wproj[:, (2 * v + 1) * D_STFT:(2 * v + 2) * D_STFT] = \
            lo.astype(np.float32)
    wproj = wproj.astype(bf16)
    nh, nl = _split_bf16(pw[128 * V + np.arange(V)])
    wnyq = np.concatenate([nh.astype(np.float32), nl.astype(np.float32)],
                          axis=1).astype(bf16)
    bh, bl = _split_bf16(pb.reshape(1, D_STFT))
    pbias = np.concatenate([bh.astype(np.float32), bl.astype(np.float32)],
                           axis=0).astype(bf16)

    embrep = np.zeros((128, D_EMB), np.float32)
    for a in range(4):
        embrep[32 * a:32 * a + V] = emb
    embrep = embrep.astype(bf16)

    bsel = [_bsel_for_half(h) for h in range(2)]

    in_maps = []
    for core in range(NCORES):
        b, h = divmod(core, 2)
        padded = np.pad(ids[b], 128, mode="reflect")
        seg = padded[LH * h:LH * h + 64 * (F - 1) + NFFT]  # (4352,)
        hf = np.zeros((128, 2 * F), np.int8)
        for c in range(2):
            idx = (64 * np.arange(F)[None, :] + 128 * c
                   + np.arange(128)[:, None])
            hf[:, c * F:(c + 1) * F] = seg[idx]
        ids_out = ids[b, LH * h:LH * (h + 1)]
        he = np.zeros((128, LH // 4), np.int8)
        tiles = ids_out.reshape(NT, 128)  # tile ti = 4q+a
        for a in range(4):
            rows = tiles[a::4]  # (8, 128), q-major
            he[32 * a:32 * a + V, :] = np.broadcast_to(
                rows.reshape(1, LH // 4), (V, LH // 4))
        in_maps.append({
            "h_frames": hf, "h_emb": he, "vfr": vfr, "vemb": vemb,
            "cw": cw, "bsel": bsel[h], "wproj": wproj, "wnyq": wnyq,
            "pbias": pbias, "embrep": embrep,
        })

    if _PROG is None:
        _PROG = _build_program()

    res = run_bass_kernel_spmd(_PROG, in_maps, core_ids=list(range(NCORES)))
    LAST_RESULT = res

    full = np.zeros((B, L, DM), np.float32)
    for core in range(NCORES):
        b, h = divmod(core, 2)
        full[b, LH * h:LH * (h + 1), :] = res.results[core]["out"]
    return full
